# revision 43
# baseline (speedup 1.0000x reference)
"""NNUE HalfKP EmbeddingBag + MLP kernel for 8 Trainium2 NeuronCores.

Strategy (data-parallel over the batch, per-core remapped tables):
  - The 128 blocks of 128 consecutive bags are greedily balanced across the
    8 cores (16 blocks each); the host permutes the block outputs back.
  - Each core's index stream is deduplicated host-side; its ~31.8k unique
    table rows are uploaded as a per-core bf16 table (unique count always
    fits the int16 gather-index range, so no low/high table split).
  - Rows are gathered with gpsimd.dma_gather in 4096-row chunks with
    single_packet=False (single-packet gathers cap at 64 descriptors per
    SDMA engine = 1024 rows and wedge the device beyond that). All chunks
    are statically full -- pads gather row 0 with bag id -1 -- so the row
    count is an immediate and no per-gather register loads are needed.
  - Each 128-bag block's rows are segment-summed into a PSUM block with
    TensorE matmuls against 0/1 selection matrices built on-device (one DVE
    is_equal per chunk via a stride-0 broadcast AP). The PSUM is seeded with
    ones^T @ bias1 so no bias rows are gathered.
  - relu -> fc2 -> relu -> out_w run on-chip; each core writes 2048 floats.
"""

import numpy as np

import concourse.bacc as bacc
import concourse.mybir as mybir
from concourse.tile import TileContext
from concourse.masks import make_identity

# ---------------- problem constants (hardcoded per spec) ----------------
NUM_FEATURES = 41024
HIDDEN = 256
FC2 = 32
BATCH = 16384
N_IDX = 491520
N_CORES = 8

BAGS_PER_CORE = BATCH // N_CORES       # 2048
BLOCK_BAGS = 128                       # bags per PSUM block
NBLK = BAGS_PER_CORE // BLOCK_BAGS     # 16 block slots per core
NBLK_G = BATCH // BLOCK_BAGS           # 128 global blocks
ROWS_PER_GATHER = 4096                 # rows per dma_gather (single_packet=False)
TILE = 128                             # rows per matmul tile
CHUNK_TILES = ROWS_PER_GATHER // TILE  # 8 tiles per gather chunk
N_QUEUES = 4
TABLE_ROWS = 32769                     # per-core unique rows (<=32768) + bias row
BIAS_ROW = TABLE_ROWS - 1


def _ceil_div(a, b):
    return -(-a // b)


def _host_prep(indices, offsets):
    """Balance blocks over cores, dedup rows per core, build the chunked
    gather schedule and per-core idx/bag blobs."""
    indices = np.asarray(indices).astype(np.int64)
    offsets = np.asarray(offsets).astype(np.int64)
    n = indices.shape[0]
    seg = np.clip(
        np.searchsorted(offsets, np.arange(n), side="right") - 1, 0, BATCH - 1
    )
    blk_bounds = np.searchsorted(seg, np.arange(0, BATCH + 1, BLOCK_BAGS))
    sizes = blk_bounds[1:] - blk_bounds[:-1]           # rows per global block

    # greedy balance: biggest block to least-loaded core (cap 16 blocks/core)
    order = np.argsort(-sizes, kind="stable")
    loads = [0] * N_CORES
    counts = [0] * N_CORES
    assign = [[] for _ in range(N_CORES)]              # core -> global block ids
    for g in order:
        c = min(
            (c for c in range(N_CORES) if counts[c] < NBLK),
            key=lambda c: loads[c],
        )
        assign[c].append(int(g))
        loads[c] += int(sizes[g])
        counts[c] += 1
    # slot-align by size rank so per-slot max over cores is tight
    for c in range(N_CORES):
        assign[c].sort(key=lambda g: -sizes[g])

    # per (core, slot): dedup within the block -- a row used by k bags of the
    # block becomes ceil(k/2) entries (rowE, bagA, bagB[-1 if unpaired]).
    # Paired entries (bagB>=0) are placed first, each group sorted by row.
    per_cs = []
    npair_cs = []
    for c in range(N_CORES):
        per_s = []
        npair_s = []
        for g in assign[c]:
            lo, hi = blk_bounds[g], blk_bounds[g + 1]
            raw = indices[lo:hi]
            bags = seg[lo:hi] - g * BLOCK_BAGS
            o = np.lexsort((bags, raw))
            r_s, b_s = raw[o], bags[o]
            uniq_r, starts, counts = np.unique(
                r_s, return_index=True, return_counts=True
            )
            ent_counts = (counts + 1) // 2
            ent_starts = np.concatenate([[0], np.cumsum(ent_counts)[:-1]])
            n_ent = int(ent_counts.sum())
            run_id = np.repeat(np.arange(len(starts)), counts)
            pos = np.arange(len(r_s)) - starts[run_id]
            ent_id = ent_starts[run_id] + pos // 2
            first = pos % 2 == 0
            rowE = np.empty(n_ent, dtype=np.int64)
            bagA = np.empty(n_ent, dtype=np.int64)
            bagB = np.full(n_ent, -1, dtype=np.int64)
            rowE[ent_id] = r_s
            bagA[ent_id[first]] = b_s[first]
            bagB[ent_id[~first]] = b_s[~first]
            paired = bagB >= 0
            order2 = np.concatenate(
                [np.flatnonzero(paired), np.flatnonzero(~paired)]
            )
            per_s.append((rowE[order2], bagA[order2], bagB[order2]))
            npair_s.append(int(paired.sum()))
        per_cs.append(per_s)
        npair_cs.append(npair_s)

    # per-slot uniform tile counts + paired-tile counts (max over cores)
    T = [0] * NBLK
    T_pair = [0] * NBLK
    for s in range(NBLK):
        for c in range(N_CORES):
            T[s] = max(T[s], _ceil_div(len(per_cs[c][s][0]), TILE))
            T_pair[s] = max(T_pair[s], _ceil_div(npair_cs[c][s], TILE))
    cumT = [0] * (NBLK + 1)
    for s in range(NBLK):
        cumT[s + 1] = cumT[s] + T[s]
    total_tiles = cumT[NBLK]
    idx_cols = total_tiles * (TILE // 16)
    # global tile index -> needs the second (bagB) is_equal-add op
    pair_flags = np.zeros(total_tiles, dtype=bool)
    for s in range(NBLK):
        pair_flags[cumT[s] : cumT[s] + T_pair[s]] = True

    # chunk schedule: (tiles_in_chunk, first_tile, idx_col_base)
    chunks = []
    t0 = 0
    while t0 < total_tiles:
        tc = min(CHUNK_TILES, total_tiles - t0)
        chunks.append((tc, t0, t0 * (TILE // 16)))
        t0 += tc
    n_gathers = len(chunks)

    # per-core dedup + blobs (all-valid gathers: pads repeat the block's last
    # row with bag -1). The per-core table is ordered by FIRST USE in the
    # gather stream and each block's rows are sorted ascending, so every
    # chunk reads monotonically increasing addresses (new rows sequentially,
    # repeats as forward scans) -- maximizes HBM row-buffer hits.
    idx_blobs, bag_blobs, uniqs = [], [], []
    for c in range(N_CORES):
        all_rows = np.concatenate([per_cs[c][s][0] for s in range(NBLK)])
        uniq, first_pos = np.unique(all_rows, return_index=True)
        assert len(uniq) <= TABLE_ROWS - 1, f"core {c}: {len(uniq)} unique rows"
        order = np.argsort(first_pos, kind="stable")
        newpos = np.empty(len(uniq), dtype=np.int64)
        newpos[order] = np.arange(len(uniq))
        uniqs.append(uniq[order])
        row_stream = np.zeros(total_tiles * TILE, dtype=np.int64)
        bagA_stream = np.full(total_tiles * TILE, -1.0, dtype=np.float64)
        bagB_stream = np.full(total_tiles * TILE, -1.0, dtype=np.float64)
        for s in range(NBLK):
            rowE, bagA, bagB = per_cs[c][s]
            loc = newpos[np.searchsorted(uniq, rowE)]
            npair = npair_cs[c][s]
            # sort paired and single groups separately by table position
            p1 = np.argsort(loc[:npair], kind="stable")
            p2 = np.argsort(loc[npair:], kind="stable")
            perm = np.concatenate([p1, npair + p2]).astype(np.int64)
            loc, bagA, bagB = loc[perm], bagA[perm], bagB[perm]
            r0 = cumT[s] * TILE
            row_stream[r0 : r0 + len(loc)] = loc
            bagA_stream[r0 : r0 + len(loc)] = bagA
            bagB_stream[r0 : r0 + len(loc)] = bagB
            if len(loc):  # keep pad reads monotone (and row-buffer hot)
                row_stream[r0 + len(loc) : (cumT[s] + T[s]) * TILE] = loc[-1]
        # wrap per chunk: row i of chunk -> [i%16, icol + i//16], replicated x8
        idx_arr = np.zeros((128, idx_cols), dtype=np.int16)
        for (tc, tb, icol) in chunks:
            rows = row_stream[tb * TILE : (tb + tc) * TILE]
            w = rows.reshape(tc * TILE // 16, 16).T.astype(np.int16)
            idx_arr[:, icol : icol + tc * TILE // 16] = np.tile(w, (8, 1))
        idx_blobs.append(idx_arr)
        bag_blobs.append(
            np.ascontiguousarray(
                np.concatenate(
                    [
                        bagA_stream.reshape(total_tiles, TILE).T,
                        bagB_stream.reshape(total_tiles, TILE).T,
                    ],
                    axis=1,
                ).astype(np.float32)
            )
        )

    return (chunks, T, cumT, total_tiles, idx_cols, n_gathers, pair_flags,
            idx_blobs, bag_blobs, uniqs, assign)


def _build_program(chunks, T, cumT, total_tiles, idx_cols, n_gathers, pair_flags,
                   reps=1):
    bf16 = mybir.dt.bfloat16
    f32 = mybir.dt.float32
    nc = bacc.Bacc(
        "TRN2", dynamic_dma_scratch_size=65536, num_swdge_queues=N_QUEUES
    )
    table = nc.dram_tensor("table", [TABLE_ROWS, HIDDEN], bf16, kind="ExternalInput")
    idx_d = nc.dram_tensor("idxs", [128, idx_cols], mybir.dt.int16, kind="ExternalInput")
    bag_d = nc.dram_tensor("bags", [128, 2 * total_tiles], f32, kind="ExternalInput")
    iota_d = nc.dram_tensor("iotar", [128, TILE], bf16, kind="ExternalInput")
    w2_d = nc.dram_tensor("w2", [HIDDEN, FC2], f32, kind="ExternalInput")   # fc2_w.T
    b2_d = nc.dram_tensor("b2", [FC2, 1], f32, kind="ExternalInput")
    w3_d = nc.dram_tensor("w3", [FC2, 1], f32, kind="ExternalInput")        # out_w.T
    b3_d = nc.dram_tensor("b3", [1, 1], f32, kind="ExternalInput")
    out_d = nc.dram_tensor("out", [1, BAGS_PER_CORE], f32, kind="ExternalOutput")

    NRING = 3
    NSEG = 3

    with TileContext(nc) as tc_:
        with (
            tc_.tile_pool(name="const", bufs=1) as cpool,
            tc_.tile_pool(name="gdst", bufs=1) as gpool,
            tc_.tile_pool(name="seg", bufs=1) as spool,
            tc_.tile_pool(name="hrel", bufs=2) as hpool,
            tc_.tile_pool(name="hTp", bufs=2) as hTpool,
            tc_.tile_pool(name="h2", bufs=2) as h2pool,
            tc_.tile_pool(name="ph", bufs=2, space="PSUM") as phpool,
            tc_.tile_pool(name="pt", bufs=2, space="PSUM") as ptpool,
            tc_.tile_pool(name="pm", bufs=2, space="PSUM") as pmpool,
        ):
            # first chunk's index columns live in their own tile so gather 0
            # starts after a 64KB DMA instead of the full 1MB idx upload
            icols0 = min(CHUNK_TILES * (TILE // 16), idx_cols)
            idx_sb0 = cpool.tile([128, icols0], mybir.dt.int16)
            idx_sb = cpool.tile([128, idx_cols], mybir.dt.int16)
            bag_sb = cpool.tile([128, 2 * total_tiles], f32)
            iota_sb = cpool.tile([128, TILE], bf16)
            w2_sb = cpool.tile([128, 2 * FC2], f32)
            b2_sb = cpool.tile([FC2, 1], f32)
            w3_sb = cpool.tile([FC2, 1], f32)
            b3_sb = cpool.tile([1, 1], f32)
            ident = cpool.tile([128, 128], f32)
            out_sb = cpool.tile([1, BAGS_PER_CORE], f32)
            ones1 = cpool.tile([1, 128], bf16)
            brow = cpool.tile([1, HIDDEN], bf16)

            nc.sync.dma_start(idx_sb0[:, :], idx_d[:, 0:icols0])
            nc.sync.dma_start(iota_sb[:, :], iota_d[:, :])
            nc.sync.dma_start(bag_sb[:, :], bag_d[:, :])
            nc.sync.dma_start(idx_sb[:, :], idx_d[:, :])
            nc.sync.dma_start(w2_sb[:, 0:FC2], w2_d[0:128, :])
            nc.sync.dma_start(w2_sb[:, FC2 : 2 * FC2], w2_d[128:256, :])
            nc.sync.dma_start(b2_sb[:, :], b2_d[:, :])
            nc.sync.dma_start(w3_sb[:, :], w3_d[:, :])
            nc.sync.dma_start(b3_sb[:, :], b3_d[:, :])
            # bias1 lives as the last table row; each block's PSUM is seeded
            # with ones^T @ bias1 so no bias rows are gathered per bag
            nc.sync.dma_start(brow[:, :], table[BIAS_ROW : BIAS_ROW + 1, :])
            nc.vector.memset(ones1[:, :], 1.0)
            make_identity(nc, ident[:, :])

            dst_ring = [
                gpool.tile([128, CHUNK_TILES * HIDDEN], bf16, name=f"dstr{i}", bufs=1)
                for i in range(NRING)
            ]
            seg_ring = [
                spool.tile([128, CHUNK_TILES * TILE], bf16, name=f"segr{i}", bufs=1)
                for i in range(NSEG)
            ]

            def one_pass():
                hT = hTpool.tile(
                    [128, 2 * BAGS_PER_CORE], f32, name="hT", tag="hT"
                )
                issued = [0]

                def issue_until(tile_limit):
                    while issued[0] < n_gathers and chunks[issued[0]][1] < tile_limit:
                        k = issued[0]
                        tc, tb, icol = chunks[k]
                        dst = dst_ring[k % NRING]
                        idx_src = (
                            idx_sb0[:, icol : icol + tc * TILE // 16]
                            if k == 0
                            else idx_sb[:, icol : icol + tc * TILE // 16]
                        )
                        nc.gpsimd.dma_gather(
                            dst[:, 0 : tc * HIDDEN].rearrange(
                                "p (t e) -> p t e", e=HIDDEN
                            ),
                            table[0 : TABLE_ROWS - 1, :],
                            idx_src,
                            tc * TILE,
                            tc * TILE,
                            HIDDEN,
                            single_packet=False,
                            queue_num=k % N_QUEUES,
                        )
                        seg = seg_ring[k % NSEG]
                        for t in range(tc):
                            nc.vector.tensor_scalar(
                                out=seg[:, t * TILE : (t + 1) * TILE],
                                in0=iota_sb[:, :],
                                scalar1=bag_sb[:, tb + t : tb + t + 1],
                                scalar2=None,
                                op0=mybir.AluOpType.is_equal,
                            )
                            if pair_flags[tb + t]:
                                nc.vector.scalar_tensor_tensor(
                                    out=seg[:, t * TILE : (t + 1) * TILE],
                                    in0=iota_sb[:, :],
                                    scalar=bag_sb[
                                        :, total_tiles + tb + t : total_tiles + tb + t + 1
                                    ],
                                    in1=seg[:, t * TILE : (t + 1) * TILE],
                                    op0=mybir.AluOpType.is_equal,
                                    op1=mybir.AluOpType.add,
                                )
                        issued[0] = k + 1

                def mlp_group(g):
                    p2 = pmpool.tile([FC2, 512], f32, name="p2t", tag="p2")
                    for half in range(2):
                        nc.tensor.matmul(
                            p2[:, :],
                            lhsT=w2_sb[:, half * FC2 : (half + 1) * FC2],
                            rhs=hT[
                                :,
                                half * BAGS_PER_CORE
                                + g * 512 : half * BAGS_PER_CORE
                                + (g + 1) * 512,
                            ],
                            start=(half == 0),
                            stop=(half == 1),
                        )
                    h2 = h2pool.tile([FC2, 512], f32, name="h2t", tag="h2")
                    nc.scalar.activation(
                        h2[:, :], p2[:, :], mybir.ActivationFunctionType.Relu,
                        bias=b2_sb[:, :],
                    )
                    p3 = pmpool.tile([1, 512], f32, name="p3t", tag="p3")
                    nc.tensor.matmul(
                        p3[:, :], lhsT=w3_sb[:, :], rhs=h2[:, :], start=True, stop=True
                    )
                    nc.vector.tensor_scalar_add(
                        out_sb[:, g * 512 : (g + 1) * 512], p3[:, :], b3_sb[0:1, 0:1]
                    )

                for s in range(NBLK):
                    issue_until(
                        min(cumT[min(s + 1, NBLK)] + CHUNK_TILES, total_tiles + 1)
                    )
                    psum = phpool.tile([128, HIDDEN], f32, name="psumh", tag="psumh")
                    # seed PSUM with bias1 broadcast to all 128 bags
                    nc.tensor.matmul(
                        psum[:, :], lhsT=ones1[:, :], rhs=brow[:, :],
                        start=True, stop=(T[s] == 0),
                    )
                    for t in range(cumT[s], cumT[s] + T[s]):
                        k, off = divmod(t, CHUNK_TILES)
                        nc.tensor.matmul(
                            psum[:, :],
                            lhsT=seg_ring[k % NSEG][:, off * TILE : (off + 1) * TILE],
                            rhs=dst_ring[k % NRING][
                                :, off * HIDDEN : (off + 1) * HIDDEN
                            ],
                            start=False,
                            stop=(t == cumT[s] + T[s] - 1),
                        )
                    hrel = hpool.tile([128, HIDDEN], f32, name="hrel", tag="hrel")
                    nc.scalar.activation(
                        hrel[:, :], psum[:, :], mybir.ActivationFunctionType.Relu
                    )
                    for half in range(2):
                        pt = ptpool.tile([128, 128], f32, name="ptt", tag="ptt")
                        nc.tensor.transpose(
                            pt[:, :],
                            hrel[:, half * 128 : (half + 1) * 128],
                            ident[:, :],
                        )
                        nc.scalar.copy(
                            hT[
                                :,
                                half * BAGS_PER_CORE
                                + s * 128 : half * BAGS_PER_CORE
                                + (s + 1) * 128,
                            ],
                            pt[:, :],
                        )
                    # run each 512-bag MLP group as soon as its 4 blocks land
                    if s % 4 == 3:
                        mlp_group(s // 4)

            for _rep in range(reps):
                one_pass()
            nc.sync.dma_start(out_d[:, :], out_sb[:, :])
    nc.compile()
    return nc


def _make_in_maps(inputs, sched_data):
    (chunks, T, cumT, total_tiles, idx_cols, n_gathers, pair_flags,
     idx_blobs, bag_blobs, uniqs, assign) = sched_data
    import ml_dtypes

    embed_weight = np.asarray(inputs["embed_weight"], dtype=np.float32)
    bias1 = np.asarray(inputs["bias1"], dtype=np.float32)
    fc2_w = np.asarray(inputs["fc2_w"], dtype=np.float32)
    fc2_b = np.asarray(inputs["fc2_b"], dtype=np.float32)
    out_w = np.asarray(inputs["out_w"], dtype=np.float32)
    out_b = np.asarray(inputs["out_b"], dtype=np.float32)

    iota_rep = np.arange(TILE, dtype=np.float32)[None, :].repeat(128, axis=0)
    common = {
        "iotar": iota_rep.astype(ml_dtypes.bfloat16),
        "w2": fc2_w.T.copy(),
        "b2": fc2_b.reshape(FC2, 1),
        "w3": out_w.reshape(1, FC2).T.copy(),
        "b3": out_b.reshape(1, 1),
    }
    in_maps = []
    for c in range(N_CORES):
        tbl = np.zeros((TABLE_ROWS, HIDDEN), dtype=np.float32)
        tbl[: len(uniqs[c])] = embed_weight[uniqs[c]]
        tbl[BIAS_ROW] = bias1
        m = dict(common)
        m["table"] = tbl.astype(ml_dtypes.bfloat16)
        m["idxs"] = idx_blobs[c]
        m["bags"] = bag_blobs[c]
        in_maps.append(m)
    return in_maps


def kernel(**inputs) -> np.ndarray:
    from concourse.bass_utils import run_bass_kernel_spmd

    sched_data = _host_prep(inputs["indices"], inputs["offsets"])
    nc = _build_program(*sched_data[:7])
    in_maps = _make_in_maps(inputs, sched_data)
    res = run_bass_kernel_spmd(nc, in_maps, core_ids=list(range(N_CORES)))
    assign = sched_data[10]
    out = np.empty(BATCH, dtype=np.float32)
    for c in range(N_CORES):
        vals = np.asarray(res.results[c]["out"]).reshape(BAGS_PER_CORE)
        for s, g in enumerate(assign[c]):
            out[g * BLOCK_BAGS : (g + 1) * BLOCK_BAGS] = vals[
                s * BLOCK_BAGS : (s + 1) * BLOCK_BAGS
            ]
    return out



# revision 48
# speedup vs baseline: 3.2382x; 3.2382x over previous
"""NNUE HalfKP EmbeddingBag + MLP kernel for 8 Trainium2 NeuronCores.

Strategy (data-parallel over the batch, per-core remapped tables):
  - The 128 blocks of 128 consecutive bags are greedily balanced across the
    8 cores (16 blocks each); the host permutes the block outputs back.
  - Each core's index stream is deduplicated host-side; its ~31.8k unique
    table rows are uploaded as a per-core bf16 table (unique count always
    fits the int16 gather-index range, so no low/high table split).
  - Rows are gathered with gpsimd.dma_gather in 4096-row chunks with
    single_packet=False (single-packet gathers cap at 64 descriptors per
    SDMA engine = 1024 rows and wedge the device beyond that). All chunks
    are statically full -- pads gather row 0 with bag id -1 -- so the row
    count is an immediate and no per-gather register loads are needed.
  - Each 128-bag block's rows are segment-summed into a PSUM block with
    TensorE matmuls against 0/1 selection matrices built on-device (one DVE
    is_equal per chunk via a stride-0 broadcast AP). The PSUM is seeded with
    ones^T @ bias1 so no bias rows are gathered.
  - relu -> fc2 -> relu -> out_w run on-chip; each core writes 2048 floats.
"""

import numpy as np

import concourse.bacc as bacc
import concourse.mybir as mybir
from concourse.tile import TileContext
from concourse.masks import make_identity

# ---------------- problem constants (hardcoded per spec) ----------------
NUM_FEATURES = 41024
HIDDEN = 256
FC2 = 32
BATCH = 16384
N_IDX = 491520
N_CORES = 8

BAGS_PER_CORE = BATCH // N_CORES       # 2048
BLOCK_BAGS = 128                       # bags per PSUM block
NBLK = BAGS_PER_CORE // BLOCK_BAGS     # 16 block slots per core
NBLK_G = BATCH // BLOCK_BAGS           # 128 global blocks
ROWS_PER_GATHER = 4096                 # rows per dma_gather (single_packet=False)
TILE = 128                             # rows per matmul tile
CHUNK_TILES = ROWS_PER_GATHER // TILE  # 8 tiles per gather chunk
N_QUEUES = 4
TABLE_ROWS = 32769                     # per-core unique rows (<=32768) + bias row
BIAS_ROW = TABLE_ROWS - 1


def _ceil_div(a, b):
    return -(-a // b)


def _host_prep(indices, offsets):
    """Balance blocks over cores, dedup rows per core, build the chunked
    gather schedule and per-core idx/bag blobs."""
    indices = np.asarray(indices).astype(np.int64)
    offsets = np.asarray(offsets).astype(np.int64)
    n = indices.shape[0]
    seg = np.clip(
        np.searchsorted(offsets, np.arange(n), side="right") - 1, 0, BATCH - 1
    )
    blk_bounds = np.searchsorted(seg, np.arange(0, BATCH + 1, BLOCK_BAGS))
    sizes = blk_bounds[1:] - blk_bounds[:-1]           # rows per global block

    # greedy balance: biggest block to least-loaded core (cap 16 blocks/core)
    order = np.argsort(-sizes, kind="stable")
    loads = [0] * N_CORES
    counts = [0] * N_CORES
    assign = [[] for _ in range(N_CORES)]              # core -> global block ids
    for g in order:
        c = min(
            (c for c in range(N_CORES) if counts[c] < NBLK),
            key=lambda c: loads[c],
        )
        assign[c].append(int(g))
        loads[c] += int(sizes[g])
        counts[c] += 1
    # slot-align by size rank so per-slot max over cores is tight
    for c in range(N_CORES):
        assign[c].sort(key=lambda g: -sizes[g])

    # per (core, slot): dedup within the block -- a row used by k bags of the
    # block becomes ceil(k/2) entries (rowE, bagA, bagB[-1 if unpaired]).
    # Paired entries (bagB>=0) are placed first, each group sorted by row.
    per_cs = []
    npair_cs = []
    for c in range(N_CORES):
        per_s = []
        npair_s = []
        for g in assign[c]:
            lo, hi = blk_bounds[g], blk_bounds[g + 1]
            raw = indices[lo:hi]
            bags = seg[lo:hi] - g * BLOCK_BAGS
            o = np.lexsort((bags, raw))
            r_s, b_s = raw[o], bags[o]
            uniq_r, starts, counts = np.unique(
                r_s, return_index=True, return_counts=True
            )
            ent_counts = (counts + 1) // 2
            ent_starts = np.concatenate([[0], np.cumsum(ent_counts)[:-1]])
            n_ent = int(ent_counts.sum())
            run_id = np.repeat(np.arange(len(starts)), counts)
            pos = np.arange(len(r_s)) - starts[run_id]
            ent_id = ent_starts[run_id] + pos // 2
            first = pos % 2 == 0
            rowE = np.empty(n_ent, dtype=np.int64)
            bagA = np.empty(n_ent, dtype=np.int64)
            bagB = np.full(n_ent, -1, dtype=np.int64)
            rowE[ent_id] = r_s
            bagA[ent_id[first]] = b_s[first]
            bagB[ent_id[~first]] = b_s[~first]
            paired = bagB >= 0
            order2 = np.concatenate(
                [np.flatnonzero(paired), np.flatnonzero(~paired)]
            )
            per_s.append((rowE[order2], bagA[order2], bagB[order2]))
            npair_s.append(int(paired.sum()))
        per_cs.append(per_s)
        npair_cs.append(npair_s)

    # per-slot uniform tile counts + paired-tile counts (max over cores)
    T = [0] * NBLK
    T_pair = [0] * NBLK
    for s in range(NBLK):
        for c in range(N_CORES):
            T[s] = max(T[s], _ceil_div(len(per_cs[c][s][0]), TILE))
            T_pair[s] = max(T_pair[s], _ceil_div(npair_cs[c][s], TILE))
    cumT = [0] * (NBLK + 1)
    for s in range(NBLK):
        cumT[s + 1] = cumT[s] + T[s]
    total_tiles = cumT[NBLK]
    idx_cols = total_tiles * (TILE // 16)
    # global tile index -> needs the second (bagB) is_equal-add op
    pair_flags = np.zeros(total_tiles, dtype=bool)
    for s in range(NBLK):
        pair_flags[cumT[s] : cumT[s] + T_pair[s]] = True

    # chunk schedule: (tiles_in_chunk, first_tile, idx_col_base)
    chunks = []
    t0 = 0
    while t0 < total_tiles:
        tc = min(CHUNK_TILES, total_tiles - t0)
        chunks.append((tc, t0, t0 * (TILE // 16)))
        t0 += tc
    n_gathers = len(chunks)

    # per-core dedup + blobs (all-valid gathers: pads repeat the block's last
    # row with bag -1). The per-core table is ordered by FIRST USE in the
    # gather stream and each block's rows are sorted ascending, so every
    # chunk reads monotonically increasing addresses (new rows sequentially,
    # repeats as forward scans) -- maximizes HBM row-buffer hits.
    idx_blobs, bag_blobs, uniqs = [], [], []
    for c in range(N_CORES):
        all_rows = np.concatenate([per_cs[c][s][0] for s in range(NBLK)])
        uniq, first_pos = np.unique(all_rows, return_index=True)
        assert len(uniq) <= TABLE_ROWS - 1, f"core {c}: {len(uniq)} unique rows"
        order = np.argsort(first_pos, kind="stable")
        newpos = np.empty(len(uniq), dtype=np.int64)
        newpos[order] = np.arange(len(uniq))
        uniqs.append(uniq[order])
        row_stream = np.zeros(total_tiles * TILE, dtype=np.int64)
        bagA_stream = np.full(total_tiles * TILE, -1.0, dtype=np.float64)
        bagB_stream = np.full(total_tiles * TILE, -1.0, dtype=np.float64)
        for s in range(NBLK):
            rowE, bagA, bagB = per_cs[c][s]
            loc = newpos[np.searchsorted(uniq, rowE)]
            npair = npair_cs[c][s]
            # sort paired and single groups separately by table position
            p1 = np.argsort(loc[:npair], kind="stable")
            p2 = np.argsort(loc[npair:], kind="stable")
            perm = np.concatenate([p1, npair + p2]).astype(np.int64)
            loc, bagA, bagB = loc[perm], bagA[perm], bagB[perm]
            r0 = cumT[s] * TILE
            row_stream[r0 : r0 + len(loc)] = loc
            bagA_stream[r0 : r0 + len(loc)] = bagA
            bagB_stream[r0 : r0 + len(loc)] = bagB
            if len(loc):  # keep pad reads monotone (and row-buffer hot)
                row_stream[r0 + len(loc) : (cumT[s] + T[s]) * TILE] = loc[-1]
        # wrap per chunk: row i of chunk -> [i%16, icol + i//16], replicated x8
        idx_arr = np.zeros((128, idx_cols), dtype=np.int16)
        for (tc, tb, icol) in chunks:
            rows = row_stream[tb * TILE : (tb + tc) * TILE]
            w = rows.reshape(tc * TILE // 16, 16).T.astype(np.int16)
            idx_arr[:, icol : icol + tc * TILE // 16] = np.tile(w, (8, 1))
        idx_blobs.append(idx_arr)
        bag_blobs.append(
            np.ascontiguousarray(
                np.concatenate(
                    [
                        bagA_stream.reshape(total_tiles, TILE).T,
                        bagB_stream.reshape(total_tiles, TILE).T,
                    ],
                    axis=1,
                ).astype(np.float32)
            )
        )

    return (chunks, T, cumT, total_tiles, idx_cols, n_gathers, pair_flags,
            idx_blobs, bag_blobs, uniqs, assign)


def _build_program(chunks, T, cumT, total_tiles, idx_cols, n_gathers, pair_flags,
                   reps=1, seg3d=False):
    bf16 = mybir.dt.bfloat16
    f32 = mybir.dt.float32
    nc = bacc.Bacc(
        "TRN2", dynamic_dma_scratch_size=65536, num_swdge_queues=N_QUEUES
    )
    table = nc.dram_tensor("table", [TABLE_ROWS, HIDDEN], bf16, kind="ExternalInput")
    idx_d = nc.dram_tensor("idxs", [128, idx_cols], mybir.dt.int16, kind="ExternalInput")
    bag_d = nc.dram_tensor("bags", [128, 2 * total_tiles], f32, kind="ExternalInput")
    iota_d = nc.dram_tensor("iotar", [128, TILE], bf16, kind="ExternalInput")
    w2_d = nc.dram_tensor("w2", [HIDDEN, FC2], f32, kind="ExternalInput")   # fc2_w.T
    b2_d = nc.dram_tensor("b2", [FC2, 1], f32, kind="ExternalInput")
    w3_d = nc.dram_tensor("w3", [FC2, 1], f32, kind="ExternalInput")        # out_w.T
    b3_d = nc.dram_tensor("b3", [1, 1], f32, kind="ExternalInput")
    out_d = nc.dram_tensor("out", [1, BAGS_PER_CORE], f32, kind="ExternalOutput")

    NRING = 3
    NSEG = 3

    with TileContext(nc) as tc_:
        with (
            tc_.tile_pool(name="const", bufs=1) as cpool,
            tc_.tile_pool(name="gdst", bufs=1) as gpool,
            tc_.tile_pool(name="seg", bufs=1) as spool,
            tc_.tile_pool(name="hrel", bufs=2) as hpool,
            tc_.tile_pool(name="hTp", bufs=2) as hTpool,
            tc_.tile_pool(name="h2", bufs=2) as h2pool,
            tc_.tile_pool(name="ph", bufs=2, space="PSUM") as phpool,
            tc_.tile_pool(name="pt", bufs=2, space="PSUM") as ptpool,
            tc_.tile_pool(name="pm", bufs=2, space="PSUM") as pmpool,
        ):
            # first chunk's index columns live in their own tile so gather 0
            # starts after a 64KB DMA instead of the full 1MB idx upload
            icols0 = min(CHUNK_TILES * (TILE // 16), idx_cols)
            idx_sb0 = cpool.tile([128, icols0], mybir.dt.int16)
            idx_sb = cpool.tile([128, idx_cols], mybir.dt.int16)
            bag_sb = cpool.tile([128, 2 * total_tiles], f32)
            iota_sb = cpool.tile([128, TILE], bf16)
            w2_sb = cpool.tile([128, 2 * FC2], f32)
            b2_sb = cpool.tile([FC2, 1], f32)
            w3_sb = cpool.tile([FC2, 1], f32)
            b3_sb = cpool.tile([1, 1], f32)
            ident = cpool.tile([128, 128], f32)
            out_sb = cpool.tile([1, BAGS_PER_CORE], f32)
            ones1 = cpool.tile([1, 128], bf16)
            brow = cpool.tile([1, HIDDEN], bf16)
            if seg3d:
                # bf16 copy of bagA + a wide iota for whole-chunk is_equal
                bag_bf = cpool.tile([128, total_tiles], bf16)
                iota_w = cpool.tile([128, CHUNK_TILES * TILE], bf16)

            nc.sync.dma_start(idx_sb0[:, :], idx_d[:, 0:icols0])
            nc.sync.dma_start(iota_sb[:, :], iota_d[:, :])
            nc.sync.dma_start(bag_sb[:, :], bag_d[:, :])
            nc.sync.dma_start(idx_sb[:, :], idx_d[:, :])
            nc.sync.dma_start(w2_sb[:, 0:FC2], w2_d[0:128, :])
            nc.sync.dma_start(w2_sb[:, FC2 : 2 * FC2], w2_d[128:256, :])
            nc.sync.dma_start(b2_sb[:, :], b2_d[:, :])
            nc.sync.dma_start(w3_sb[:, :], w3_d[:, :])
            nc.sync.dma_start(b3_sb[:, :], b3_d[:, :])
            # bias1 lives as the last table row; each block's PSUM is seeded
            # with ones^T @ bias1 so no bias rows are gathered per bag
            nc.sync.dma_start(brow[:, :], table[BIAS_ROW : BIAS_ROW + 1, :])
            nc.vector.memset(ones1[:, :], 1.0)
            make_identity(nc, ident[:, :])
            if seg3d:
                nc.vector.tensor_copy(bag_bf[:, :], bag_sb[:, 0:total_tiles])
                for t in range(CHUNK_TILES):
                    nc.vector.tensor_copy(
                        iota_w[:, t * TILE : (t + 1) * TILE], iota_sb[:, :]
                    )

            dst_ring = [
                gpool.tile([128, CHUNK_TILES * HIDDEN], bf16, name=f"dstr{i}", bufs=1)
                for i in range(NRING)
            ]
            seg_ring = [
                spool.tile([128, CHUNK_TILES * TILE], bf16, name=f"segr{i}", bufs=1)
                for i in range(NSEG)
            ]

            def one_pass():
                hT = hTpool.tile(
                    [128, 2 * BAGS_PER_CORE], f32, name="hT", tag="hT"
                )
                issued = [0]

                def issue_until(tile_limit):
                    while issued[0] < n_gathers and chunks[issued[0]][1] < tile_limit:
                        k = issued[0]
                        tc, tb, icol = chunks[k]
                        dst = dst_ring[k % NRING]
                        idx_src = (
                            idx_sb0[:, icol : icol + tc * TILE // 16]
                            if k == 0
                            else idx_sb[:, icol : icol + tc * TILE // 16]
                        )
                        nc.gpsimd.dma_gather(
                            dst[:, 0 : tc * HIDDEN].rearrange(
                                "p (t e) -> p t e", e=HIDDEN
                            ),
                            table[0 : TABLE_ROWS - 1, :],
                            idx_src,
                            tc * TILE,
                            tc * TILE,
                            HIDDEN,
                            single_packet=False,
                            queue_num=k % N_QUEUES,
                        )
                        seg = seg_ring[k % NSEG]
                        if seg3d:
                            nc.vector.tensor_tensor(
                                out=seg[:, 0 : tc * TILE].rearrange(
                                    "p (t j) -> p t j", j=TILE
                                ),
                                in0=iota_w[:, 0 : tc * TILE].rearrange(
                                    "p (t j) -> p t j", j=TILE
                                ),
                                in1=bag_bf[:, tb : tb + tc].to_broadcast(
                                    [128, tc, TILE]
                                ),
                                op=mybir.AluOpType.is_equal,
                            )
                        for t in range(tc):
                            if not seg3d:
                                nc.vector.tensor_scalar(
                                    out=seg[:, t * TILE : (t + 1) * TILE],
                                    in0=iota_sb[:, :],
                                    scalar1=bag_sb[:, tb + t : tb + t + 1],
                                    scalar2=None,
                                    op0=mybir.AluOpType.is_equal,
                                )
                            if pair_flags[tb + t]:
                                nc.vector.scalar_tensor_tensor(
                                    out=seg[:, t * TILE : (t + 1) * TILE],
                                    in0=iota_sb[:, :],
                                    scalar=bag_sb[
                                        :, total_tiles + tb + t : total_tiles + tb + t + 1
                                    ],
                                    in1=seg[:, t * TILE : (t + 1) * TILE],
                                    op0=mybir.AluOpType.is_equal,
                                    op1=mybir.AluOpType.add,
                                )
                        issued[0] = k + 1

                def mlp_group(g):
                    p2 = pmpool.tile([FC2, 512], f32, name="p2t", tag="p2")
                    for half in range(2):
                        nc.tensor.matmul(
                            p2[:, :],
                            lhsT=w2_sb[:, half * FC2 : (half + 1) * FC2],
                            rhs=hT[
                                :,
                                half * BAGS_PER_CORE
                                + g * 512 : half * BAGS_PER_CORE
                                + (g + 1) * 512,
                            ],
                            start=(half == 0),
                            stop=(half == 1),
                        )
                    h2 = h2pool.tile([FC2, 512], f32, name="h2t", tag="h2")
                    nc.scalar.activation(
                        h2[:, :], p2[:, :], mybir.ActivationFunctionType.Relu,
                        bias=b2_sb[:, :],
                    )
                    p3 = pmpool.tile([1, 512], f32, name="p3t", tag="p3")
                    nc.tensor.matmul(
                        p3[:, :], lhsT=w3_sb[:, :], rhs=h2[:, :], start=True, stop=True
                    )
                    nc.vector.tensor_scalar_add(
                        out_sb[:, g * 512 : (g + 1) * 512], p3[:, :], b3_sb[0:1, 0:1]
                    )

                for s in range(NBLK):
                    issue_until(
                        min(cumT[min(s + 1, NBLK)] + CHUNK_TILES, total_tiles + 1)
                    )
                    psum = phpool.tile([128, HIDDEN], f32, name="psumh", tag="psumh")
                    # seed PSUM with bias1 broadcast to all 128 bags
                    nc.tensor.matmul(
                        psum[:, :], lhsT=ones1[:, :], rhs=brow[:, :],
                        start=True, stop=(T[s] == 0),
                    )
                    for t in range(cumT[s], cumT[s] + T[s]):
                        k, off = divmod(t, CHUNK_TILES)
                        nc.tensor.matmul(
                            psum[:, :],
                            lhsT=seg_ring[k % NSEG][:, off * TILE : (off + 1) * TILE],
                            rhs=dst_ring[k % NRING][
                                :, off * HIDDEN : (off + 1) * HIDDEN
                            ],
                            start=False,
                            stop=(t == cumT[s] + T[s] - 1),
                        )
                    hrel = hpool.tile([128, HIDDEN], f32, name="hrel", tag="hrel")
                    nc.scalar.activation(
                        hrel[:, :], psum[:, :], mybir.ActivationFunctionType.Relu
                    )
                    for half in range(2):
                        pt = ptpool.tile([128, 128], f32, name="ptt", tag="ptt")
                        nc.tensor.transpose(
                            pt[:, :],
                            hrel[:, half * 128 : (half + 1) * 128],
                            ident[:, :],
                        )
                        nc.scalar.copy(
                            hT[
                                :,
                                half * BAGS_PER_CORE
                                + s * 128 : half * BAGS_PER_CORE
                                + (s + 1) * 128,
                            ],
                            pt[:, :],
                        )
                    # run each 512-bag MLP group as soon as its 4 blocks land
                    if s % 4 == 3:
                        mlp_group(s // 4)

            for _rep in range(reps):
                one_pass()
            nc.sync.dma_start(out_d[:, :], out_sb[:, :])
    nc.compile()
    return nc


def _make_in_maps(inputs, sched_data):
    (chunks, T, cumT, total_tiles, idx_cols, n_gathers, pair_flags,
     idx_blobs, bag_blobs, uniqs, assign) = sched_data
    import ml_dtypes

    embed_weight = np.asarray(inputs["embed_weight"], dtype=np.float32)
    bias1 = np.asarray(inputs["bias1"], dtype=np.float32)
    fc2_w = np.asarray(inputs["fc2_w"], dtype=np.float32)
    fc2_b = np.asarray(inputs["fc2_b"], dtype=np.float32)
    out_w = np.asarray(inputs["out_w"], dtype=np.float32)
    out_b = np.asarray(inputs["out_b"], dtype=np.float32)

    iota_rep = np.arange(TILE, dtype=np.float32)[None, :].repeat(128, axis=0)
    common = {
        "iotar": iota_rep.astype(ml_dtypes.bfloat16),
        "w2": fc2_w.T.copy(),
        "b2": fc2_b.reshape(FC2, 1),
        "w3": out_w.reshape(1, FC2).T.copy(),
        "b3": out_b.reshape(1, 1),
    }
    in_maps = []
    for c in range(N_CORES):
        tbl = np.zeros((TABLE_ROWS, HIDDEN), dtype=np.float32)
        tbl[: len(uniqs[c])] = embed_weight[uniqs[c]]
        tbl[BIAS_ROW] = bias1
        m = dict(common)
        m["table"] = tbl.astype(ml_dtypes.bfloat16)
        m["idxs"] = idx_blobs[c]
        m["bags"] = bag_blobs[c]
        in_maps.append(m)
    return in_maps


def kernel(**inputs) -> np.ndarray:
    from concourse.bass_utils import run_bass_kernel_spmd

    sched_data = _host_prep(inputs["indices"], inputs["offsets"])
    nc = _build_program(*sched_data[:7], seg3d=True)
    in_maps = _make_in_maps(inputs, sched_data)
    res = run_bass_kernel_spmd(nc, in_maps, core_ids=list(range(N_CORES)))
    assign = sched_data[10]
    out = np.empty(BATCH, dtype=np.float32)
    for c in range(N_CORES):
        vals = np.asarray(res.results[c]["out"]).reshape(BAGS_PER_CORE)
        for s, g in enumerate(assign[c]):
            out[g * BLOCK_BAGS : (g + 1) * BLOCK_BAGS] = vals[
                s * BLOCK_BAGS : (s + 1) * BLOCK_BAGS
            ]
    return out

